# revision 8
# baseline (speedup 1.0000x reference)
"""Trainium2 Bass kernel for GPT-Neo style causal attention.

reference:
    scores = q @ k.T              (no 1/sqrt(d) scaling), fp32
    scores = where(causal, scores, -inf)
    attn   = softmax(scores, -1)
    attn   = attn * ctx_mask[b, None, None, :]
    out    = attn @ v

Shapes: B=2, H=16, S=2048, D=128 fp32. Sharded over 8 cores by (b*h) —
4 heads per core; each core's heads belong to one batch, so one
ctx_mask row per core.

Per-core algorithm (T-layout softmax: k on partitions, q on free axis):
  - Q,K are cast to fp16 (DVE) and transposed by the DMA XBAR (one
    whole-tensor dma_start_transpose per head-tensor) -> no PE
    transposes, no fp32-family LDWEIGHTS anywhere.
  - per key-block t: scoresT[k,q] = KT_t.T @ QT (fp16, 1 cyc/col; FWL
    weight loads hide under the 512-col streams).
  - one exp() per strip on ScalarE with per-partition bias ln(ctx_mask):
    expT = exp(s - 16 + ln(cm_key)) = exp(s)*cm_key -> bf16 (the
    ctx-mask multiply costs nothing).  Causal diagonal block masked
    post-exp by a 0/1 upper-tri multiply on DVE (bf16; exp stays finite
    for |s| < 104).
  - AV: out_psum[q, 0:129] = sum_kb expT_kb.T @ [V | 1/cm] (bf16,
    fp32 PSUM accum).  Column 128 accumulates exp*cm*(1/cm) = exp,
    i.e. the pre-ctx-mask softmax denominator -> reciprocal + scale.
  - cm clamped at 1e-30 so cm=0 stays exact.

Scheduling: per-head input loads are serialized by a single-buffered
stage pool (full DMA bandwidth for the head at the front), and each
head's cast/transpose/vp prologue is emitted one head ahead of the
previous head's compute loop so the Tile scheduler overlaps them.
A dummy bf16 matmul burst warms the PE HAM clock gate to 2.4 GHz while
the first inputs load.
"""

from contextlib import ExitStack

import numpy as np

import concourse.bass as bass
import concourse.mybir as mybir
import concourse.tile as tile
from concourse.bass_utils import run_bass_kernel_spmd
from concourse.masks import make_upper_triangular

F32 = mybir.dt.float32
F16 = mybir.dt.float16
BF16 = mybir.dt.bfloat16

B, H, S, D = 2, 16, 2048, 128
NCORES = 8
NBH = (B * H) // NCORES  # heads per core


def _legalize_waits(nc):
    """This container's walrus accepts at most 1 sync wait per instruction
    (2 for EventSemaphore). Hoist extra waits onto same-engine NoOps
    inserted immediately before the offending instruction (semantically
    identical: all waits still complete before it executes)."""
    n = 0
    ctr = [0]
    for f in nc.m.functions:
        for bb in f.blocks:
            out = []
            dirty = False
            for inst in bb.instructions:
                si = inst.sync_info
                cap = 2 if isinstance(inst, mybir.InstEventSemaphore) else 1
                if si is not None and len(si.on_wait) > cap:
                    waits = list(si.on_wait)
                    extra, keep = waits[:-cap], waits[-cap:]
                    for w in extra:
                        ctr[0] += 1
                        nop = mybir.InstNoOp(
                            name=f"waitsplit-{ctr[0]}",
                            ins=[],
                            outs=[],
                            engine=inst.engine,
                            sync_info=mybir.SyncInfo(on_wait=[w], on_update=[]),
                        )
                        nc.register_instruction(nop, overwrite=True)
                        out.append(nop)
                    inst.sync_info = mybir.SyncInfo(
                        on_wait=keep, on_update=list(si.on_update)
                    )
                    dirty = True
                    n += 1
                out.append(inst)
            if dirty:
                bb.instructions = out
    return n


def build_nc(nbh=NBH, s=S, d=D, num_devices=NCORES):
    SB = s // 128  # 128-row blocks along the sequence
    nc = bass.Bass("TRN2", target_bir_lowering=False, debug=False,
                   num_devices=num_devices)
    q = nc.dram_tensor("q", [nbh, s, d], F32, kind="ExternalInput")
    k = nc.dram_tensor("k", [nbh, s, d], F32, kind="ExternalInput")
    v = nc.dram_tensor("v", [nbh, s, d], F32, kind="ExternalInput")
    cm = nc.dram_tensor("cm", [s], F32, kind="ExternalInput")
    o = nc.dram_tensor("out", [nbh, s, d], F32, kind="ExternalOutput")

    EXPFN = mybir.ActivationFunctionType.Exp
    LNFN = mybir.ActivationFunctionType.Ln

    with tile.TileContext(nc) as tc, ExitStack() as ctx:
        consts = ctx.enter_context(tc.tile_pool(name="consts", bufs=1))
        stage = ctx.enter_context(tc.tile_pool(name="stage", bufs=1))
        h16 = ctx.enter_context(tc.tile_pool(name="h16", bufs=2))
        tpool = ctx.enter_context(tc.tile_pool(name="tpool", bufs=2))
        vpool = ctx.enter_context(tc.tile_pool(name="vpool", bufs=2))
        epool = ctx.enter_context(tc.tile_pool(name="epool", bufs=1))
        opool = ctx.enter_context(tc.tile_pool(name="opool", bufs=2))
        small = ctx.enter_context(tc.tile_pool(name="small", bufs=4))
        psum = ctx.enter_context(tc.tile_pool(name="psum", bufs=2, space="PSUM"))
        psav = ctx.enter_context(tc.tile_pool(name="psav", bufs=2, space="PSUM"))

        # 0/1 upper-triangular (incl diag) keep-mask for the causal
        # diagonal block, applied to expT (post-exp) in bf16.
        tri32 = consts.tile([128, 128], F32)
        make_upper_triangular(nc, tri32, val=1.0, diag=True)
        tri = consts.tile([128, 128], BF16)
        nc.vector.tensor_copy(tri, tri32)

        # ctx-mask pipeline: cmc = max(cm, 1e-30); lncm = ln(cmc) - 16
        # (exp bias); invc = 1/cmc in bf16 (denominator column of V')
        cmt = consts.tile([128, SB], F32)
        nc.sync.dma_start(out=cmt, in_=cm.ap().rearrange("(sb p) -> p sb", p=128))
        cmc = consts.tile([128, SB], F32)
        nc.vector.tensor_scalar_max(cmc, cmt, 1e-30)
        lncm = consts.tile([128, SB], F32)
        nc.scalar.activation(lncm, cmc, LNFN)
        nc.vector.tensor_scalar_add(lncm, lncm, -16.0)
        invc = consts.tile([128, SB], F32)
        nc.vector.reciprocal(invc, cmc)
        invcb = consts.tile([128, SB], BF16)
        nc.vector.tensor_copy(invcb, invc)

        # Dummy bf16 matmuls (values irrelevant) to warm the PE clock gate
        # while the first input DMAs + casts + transposes are in flight.
        wpw = consts.tile([128, 128], BF16)
        nc.vector.memset(wpw, 1.0)
        wps = psav.tile([128, 256], F32, tag="av")
        for _ in range(150):
            nc.tensor.matmul(wps[:, 0:128], wpw, wpw, start=True, stop=True)

        qap, kap, vap, oap = q.ap(), k.ap(), v.ap(), o.ap()

        def loads(bh):
            qn = stage.tile([128, SB, d], F32, tag="qn")
            kn = stage.tile([128, SB, d], F32, tag="kn")
            vn = stage.tile([128, SB, d], F32, tag="vn")
            nc.sync.dma_start(out=qn, in_=qap[bh].rearrange("(sb p) d -> p sb d", p=128))
            nc.sync.dma_start(out=kn, in_=kap[bh].rearrange("(sb p) d -> p sb d", p=128))
            nc.sync.dma_start(out=vn, in_=vap[bh].rearrange("(sb p) d -> p sb d", p=128))
            return qn, kn, vn

        def mid(bh, qn, kn, vn):
            # fp16 casts feeding the XBAR transposes
            q16 = h16.tile([128, SB, d], F16, tag="q16")
            k16 = h16.tile([128, SB, d], F16, tag="k16")
            nc.vector.tensor_copy(q16, qn)
            nc.vector.tensor_copy(k16, kn)
            # whole-tensor DMA XBAR transposes: qt[dcol, sb, qrow] = Q^T
            qt = tpool.tile([128, SB, 128], F16, tag="qt")
            kt = tpool.tile([128, SB, 128], F16, tag="kt")
            nc.sync.dma_start_transpose(out=qt, in_=q16)
            nc.sync.dma_start_transpose(out=kt, in_=k16)
            # V' = [V | 1/cm] bf16
            vp = vpool.tile([128, SB, d + 1], BF16, tag="vp")
            nc.vector.tensor_copy(vp[:, :, 0:d], vn)
            nc.vector.tensor_copy(vp[:, :, d], invcb)
            return qt, kt, vp

        def tloop(bh, qt, kt, vp, emit_next):
            expT = epool.tile([128, SB, s], BF16, tag="expT",
                              name=f"expT_{bh}")
            ostage = opool.tile([128, SB, d], F32, tag="ostage")

            def av_block(qb):
                av = psav.tile([128, 256], F32, tag="av")
                for kb in range(qb + 1):
                    nc.tensor.matmul(
                        av[:, 0:d + 1],
                        expT[:, kb, qb * 128:(qb + 1) * 128],
                        vp[:, kb, :],
                        start=(kb == 0),
                        stop=(kb == qb),
                    )
                rec = small.tile([128, 1], F32, tag="rec")
                nc.vector.reciprocal(rec, av[:, d:d + 1])
                nc.vector.tensor_scalar_mul(ostage[:, qb, :], av[:, 0:d], rec)

            # scores strips capped at 1536 cols (3 PSUM banks) so two strip
            # slots + the av pool fit in the 8 PSUM banks; the long
            # strips (t < 4) are split into two slots/exps.
            for t in range(SB):
                for (lo, hi) in (((t * 128) // 512 * 512,
                                  min(((t * 128) // 512 * 512) + 1536, s)),
                                 (min(((t * 128) // 512 * 512) + 1536, s), s)):
                    if lo >= hi:
                        continue
                    sc = psum.tile([128, 1536], F32, tag="ps")
                    qstart = max(t * 128, lo)
                    while qstart < hi:
                        seg = min(512 - (qstart % 512), hi - qstart)
                        b0, b1 = qstart // 128, (qstart + seg) // 128
                        nc.tensor.matmul(
                            sc[:, qstart - lo:qstart - lo + seg],
                            kt[:, t, :],
                            qt[:, b0:b1, :],
                            start=True,
                            stop=True,
                        )
                        qstart += seg
                    q0 = max(t * 128, lo)
                    # exp(s - 16 + ln(cm_key)) -> bf16
                    nc.scalar.activation(expT[:, t, q0:hi], sc[:, q0 - lo:hi - lo],
                                         EXPFN, bias=lncm[:, t:t + 1])
                # causal-mask the diagonal block post-exp (0/1 multiply);
                # only the last (kb==qb) AV pair of av_block(t) waits on it
                nc.vector.tensor_mul(expT[:, t, t * 128:(t + 1) * 128],
                                     expT[:, t, t * 128:(t + 1) * 128], tri)
                if t >= 1:
                    av_block(t - 1)  # one step behind so PE never waits on exp
                if t == 5 and emit_next is not None:
                    # next head's cast/transpose/vp prologue, emitted
                    # mid-loop: its input loads have completed by now, so
                    # it slots into engine queues without blocking them
                    emit_next()
            av_block(SB - 1)

            # chunked stores on the idle GpSimd SWDGE queue: all but the
            # last chunk overlap compute, and the Sync queue stays free
            # for the next head's loads/transposes
            for g0 in range(0, SB, 4):
                gs = min(4, SB - g0)
                nc.gpsimd.dma_start(
                    out=oap[bh][g0 * 128:(g0 + gs) * 128].rearrange(
                        "(sb p) d -> p sb d", p=128),
                    in_=ostage[:, g0:g0 + gs, :],
                )

        hnd = {0: loads(0)}
        mids = {0: mid(0, *hnd[0])}
        for bh in range(nbh):
            if bh + 1 < nbh:
                hnd[bh + 1] = loads(bh + 1)

                def emit_next(b=bh + 1):
                    mids[b] = mid(b, *hnd[b])
            else:
                emit_next = None
            tloop(bh, *mids[bh], emit_next)

    _legalize_waits(nc)
    return nc


_nc_cache = {}


def _get_nc():
    key = (NBH, S, D)
    if key not in _nc_cache:
        _nc_cache[key] = build_nc()
    return _nc_cache[key]


def kernel(query, key, value, ctx_mask):
    q = np.ascontiguousarray(query, dtype=np.float32).reshape(B * H, S, D)
    k = np.ascontiguousarray(key, dtype=np.float32).reshape(B * H, S, D)
    v = np.ascontiguousarray(value, dtype=np.float32).reshape(B * H, S, D)
    cmf = np.ascontiguousarray(ctx_mask, dtype=np.float32)

    in_maps = []
    for c in range(NCORES):
        lo = c * NBH
        in_maps.append({
            "q": q[lo:lo + NBH],
            "k": k[lo:lo + NBH],
            "v": v[lo:lo + NBH],
            "cm": cmf[(lo // H)],
        })
    nc = _get_nc()
    res = run_bass_kernel_spmd(nc, in_maps, list(range(NCORES)))
    outs = [res.results[c]["out"] for c in range(NCORES)]
    return np.concatenate(outs, axis=0).reshape(B, H, S, D).astype(np.float32)


# revision 10
# speedup vs baseline: 1.0627x; 1.0627x over previous
"""Trainium2 Bass kernel for GPT-Neo style causal attention.

reference:
    scores = q @ k.T              (no 1/sqrt(d) scaling), fp32
    scores = where(causal, scores, -inf)
    attn   = softmax(scores, -1)
    attn   = attn * ctx_mask[b, None, None, :]
    out    = attn @ v

Shapes: B=2, H=16, S=2048, D=128 fp32. Sharded over 8 cores by (b*h) —
4 heads per core; each core's heads belong to one batch, so one
ctx_mask row per core.

Per-core algorithm (T-layout softmax: k on partitions, q on free axis):
  - Q,K are cast to fp16 (DVE) and transposed by the DMA XBAR (one
    whole-tensor dma_start_transpose per head-tensor) -> no PE
    transposes, no fp32-family LDWEIGHTS anywhere.
  - per key-block t: scoresT[k,q] = KT_t.T @ QT (fp16, 1 cyc/col; FWL
    weight loads hide under the 512-col streams).
  - one exp() per strip on ScalarE with per-partition bias ln(ctx_mask):
    expT = exp(s - 16 + ln(cm_key)) = exp(s)*cm_key -> bf16 (the
    ctx-mask multiply costs nothing).  Causal diagonal block masked
    post-exp by a 0/1 upper-tri multiply on DVE (bf16; exp stays finite
    for |s| < 104).
  - AV: out_psum[q, 0:129] = sum_kb expT_kb.T @ [V | 1/cm] (bf16,
    fp32 PSUM accum).  Column 128 accumulates exp*cm*(1/cm) = exp,
    i.e. the pre-ctx-mask softmax denominator -> reciprocal + scale.
  - cm clamped at 1e-30 so cm=0 stays exact.

Scheduling: per-head input loads are serialized by a single-buffered
stage pool (full DMA bandwidth for the head at the front), and each
head's cast/transpose/vp prologue is emitted one head ahead of the
previous head's compute loop so the Tile scheduler overlaps them.
A dummy bf16 matmul burst warms the PE HAM clock gate to 2.4 GHz while
the first inputs load.
"""

from contextlib import ExitStack

import numpy as np

import concourse.bass as bass
import concourse.mybir as mybir
import concourse.tile as tile
from concourse.bass_utils import run_bass_kernel_spmd
from concourse.masks import make_upper_triangular

F32 = mybir.dt.float32
F16 = mybir.dt.float16
BF16 = mybir.dt.bfloat16

B, H, S, D = 2, 16, 2048, 128
NCORES = 8
NBH = (B * H) // NCORES  # heads per core


def _legalize_waits(nc):
    """This container's walrus accepts at most 1 sync wait per instruction
    (2 for EventSemaphore). Hoist extra waits onto same-engine NoOps
    inserted immediately before the offending instruction (semantically
    identical: all waits still complete before it executes)."""
    n = 0
    ctr = [0]
    for f in nc.m.functions:
        for bb in f.blocks:
            out = []
            dirty = False
            for inst in bb.instructions:
                si = inst.sync_info
                cap = 2 if isinstance(inst, mybir.InstEventSemaphore) else 1
                if si is not None and len(si.on_wait) > cap:
                    waits = list(si.on_wait)
                    extra, keep = waits[:-cap], waits[-cap:]
                    for w in extra:
                        ctr[0] += 1
                        nop = mybir.InstNoOp(
                            name=f"waitsplit-{ctr[0]}",
                            ins=[],
                            outs=[],
                            engine=inst.engine,
                            sync_info=mybir.SyncInfo(on_wait=[w], on_update=[]),
                        )
                        nc.register_instruction(nop, overwrite=True)
                        out.append(nop)
                    inst.sync_info = mybir.SyncInfo(
                        on_wait=keep, on_update=list(si.on_update)
                    )
                    dirty = True
                    n += 1
                out.append(inst)
            if dirty:
                bb.instructions = out
    return n


def build_nc(nbh=NBH, s=S, d=D, num_devices=NCORES):
    SB = s // 128  # 128-row blocks along the sequence
    nc = bass.Bass("TRN2", target_bir_lowering=False, debug=False,
                   num_devices=num_devices)
    q = nc.dram_tensor("q", [nbh, s, d], F32, kind="ExternalInput")
    k = nc.dram_tensor("k", [nbh, s, d], F32, kind="ExternalInput")
    v = nc.dram_tensor("v", [nbh, s, d], F32, kind="ExternalInput")
    cm = nc.dram_tensor("cm", [s], F32, kind="ExternalInput")
    o = nc.dram_tensor("out", [nbh, s, d], F32, kind="ExternalOutput")

    EXPFN = mybir.ActivationFunctionType.Exp
    LNFN = mybir.ActivationFunctionType.Ln

    with tile.TileContext(nc) as tc, ExitStack() as ctx:
        consts = ctx.enter_context(tc.tile_pool(name="consts", bufs=1))
        stage = ctx.enter_context(tc.tile_pool(name="stage", bufs=2))
        h16 = ctx.enter_context(tc.tile_pool(name="h16", bufs=2))
        tpool = ctx.enter_context(tc.tile_pool(name="tpool", bufs=2))
        vpool = ctx.enter_context(tc.tile_pool(name="vpool", bufs=2))
        epool = ctx.enter_context(tc.tile_pool(name="epool", bufs=1))
        opool = ctx.enter_context(tc.tile_pool(name="opool", bufs=2))
        small = ctx.enter_context(tc.tile_pool(name="small", bufs=4))
        psum = ctx.enter_context(tc.tile_pool(name="psum", bufs=2, space="PSUM"))
        psav = ctx.enter_context(tc.tile_pool(name="psav", bufs=2, space="PSUM"))

        # 0/1 upper-triangular (incl diag) keep-mask for the causal
        # diagonal block, applied to expT (post-exp) in bf16.
        tri32 = consts.tile([128, 128], F32)
        make_upper_triangular(nc, tri32, val=1.0, diag=True)
        tri = consts.tile([128, 128], BF16)
        nc.vector.tensor_copy(tri, tri32)

        # ctx-mask pipeline: cmc = max(cm, 1e-30); lncm = ln(cmc) - 16
        # (exp bias); invc = 1/cmc in bf16 (denominator column of V')
        cmt = consts.tile([128, SB], F32)
        nc.sync.dma_start(out=cmt, in_=cm.ap().rearrange("(sb p) -> p sb", p=128))
        cmc = consts.tile([128, SB], F32)
        nc.vector.tensor_scalar_max(cmc, cmt, 1e-30)
        lncm = consts.tile([128, SB], F32)
        nc.scalar.activation(lncm, cmc, LNFN)
        nc.vector.tensor_scalar_add(lncm, lncm, -16.0)
        invc = consts.tile([128, SB], F32)
        nc.vector.reciprocal(invc, cmc)
        invcb = consts.tile([128, SB], BF16)
        nc.vector.tensor_copy(invcb, invc)

        # Dummy bf16 matmuls (values irrelevant) to warm the PE clock gate
        # while the first input DMAs + casts + transposes are in flight.
        wpw = consts.tile([128, 128], BF16)
        nc.vector.memset(wpw, 1.0)
        wps = psav.tile([128, 256], F32, tag="av")
        for _ in range(150):
            nc.tensor.matmul(wps[:, 0:128], wpw, wpw, start=True, stop=True)

        qap, kap, vap, oap = q.ap(), k.ap(), v.ap(), o.ap()

        def loads(bh):
            qn = stage.tile([128, SB, d], F32, tag="qn")
            kn = stage.tile([128, SB, d], F32, tag="kn")
            vn = stage.tile([128, SB, d], F32, tag="vn")
            nc.sync.dma_start(out=qn, in_=qap[bh].rearrange("(sb p) d -> p sb d", p=128))
            nc.sync.dma_start(out=kn, in_=kap[bh].rearrange("(sb p) d -> p sb d", p=128))
            nc.sync.dma_start(out=vn, in_=vap[bh].rearrange("(sb p) d -> p sb d", p=128))
            return qn, kn, vn

        def mid(bh, qn, kn, vn):
            # fp16 casts feeding the XBAR transposes
            q16 = h16.tile([128, SB, d], F16, tag="q16")
            k16 = h16.tile([128, SB, d], F16, tag="k16")
            nc.vector.tensor_copy(q16, qn)
            nc.vector.tensor_copy(k16, kn)
            # whole-tensor DMA XBAR transposes: qt[dcol, sb, qrow] = Q^T.
            # For head 0 (exposed lead-in) run the two descriptor
            # generations on both HWDGE queues in parallel.
            qt = tpool.tile([128, SB, 128], F16, tag="qt")
            kt = tpool.tile([128, SB, 128], F16, tag="kt")
            nc.sync.dma_start_transpose(out=qt, in_=q16)
            if bh == 0:
                nc.scalar.dma_start_transpose(out=kt, in_=k16)
            else:
                nc.sync.dma_start_transpose(out=kt, in_=k16)
            # V' = [V | 1/cm] bf16
            vp = vpool.tile([128, SB, d + 1], BF16, tag="vp")
            nc.vector.tensor_copy(vp[:, :, 0:d], vn)
            nc.vector.tensor_copy(vp[:, :, d], invcb)
            return qt, kt, vp

        def tloop(bh, qt, kt, vp, emit_next):
            expT = epool.tile([128, SB, s], BF16, tag="expT",
                              name=f"expT_{bh}")
            ostage = opool.tile([128, SB, d], F32, tag="ostage")

            def av_block(qb):
                av = psav.tile([128, 256], F32, tag="av")
                for kb in range(qb + 1):
                    nc.tensor.matmul(
                        av[:, 0:d + 1],
                        expT[:, kb, qb * 128:(qb + 1) * 128],
                        vp[:, kb, :],
                        start=(kb == 0),
                        stop=(kb == qb),
                    )
                rec = small.tile([128, 1], F32, tag="rec")
                nc.vector.reciprocal(rec, av[:, d:d + 1])
                nc.vector.tensor_scalar_mul(ostage[:, qb, :], av[:, 0:d], rec)

            # scores strips capped at 1536 cols (3 PSUM banks) so two strip
            # slots + the av pool fit in the 8 PSUM banks; the long
            # strips (t < 4) are split into two slots/exps.
            for t in range(SB):
                for (lo, hi) in (((t * 128) // 512 * 512,
                                  min(((t * 128) // 512 * 512) + 1536, s)),
                                 (min(((t * 128) // 512 * 512) + 1536, s), s)):
                    if lo >= hi:
                        continue
                    sc = psum.tile([128, 1536], F32, tag="ps")
                    qstart = max(t * 128, lo)
                    while qstart < hi:
                        seg = min(512 - (qstart % 512), hi - qstart)
                        b0, b1 = qstart // 128, (qstart + seg) // 128
                        nc.tensor.matmul(
                            sc[:, qstart - lo:qstart - lo + seg],
                            kt[:, t, :],
                            qt[:, b0:b1, :],
                            start=True,
                            stop=True,
                        )
                        qstart += seg
                    q0 = max(t * 128, lo)
                    # exp(s - 16 + ln(cm_key)) -> bf16
                    nc.scalar.activation(expT[:, t, q0:hi], sc[:, q0 - lo:hi - lo],
                                         EXPFN, bias=lncm[:, t:t + 1])
                # causal-mask the diagonal block post-exp (0/1 multiply);
                # only the last (kb==qb) AV pair of av_block(t) waits on it
                nc.vector.tensor_mul(expT[:, t, t * 128:(t + 1) * 128],
                                     expT[:, t, t * 128:(t + 1) * 128], tri)
                if t >= 1:
                    av_block(t - 1)  # one step behind so PE never waits on exp
                if t == 5 and emit_next is not None:
                    # next head's cast/transpose/vp prologue, emitted
                    # mid-loop: its input loads have completed by now, so
                    # it slots into engine queues without blocking them
                    emit_next()
            av_block(SB - 1)

            # chunked stores on the idle GpSimd SWDGE queue: all but the
            # last chunk overlap compute, and the Sync queue stays free
            # for the next head's loads/transposes
            for g0 in range(0, SB, 4):
                gs = min(4, SB - g0)
                nc.gpsimd.dma_start(
                    out=oap[bh][g0 * 128:(g0 + gs) * 128].rearrange(
                        "(sb p) d -> p sb d", p=128),
                    in_=ostage[:, g0:g0 + gs, :],
                )

        hnd = {0: loads(0)}
        mids = {0: mid(0, *hnd[0])}
        for bh in range(nbh):
            if bh + 1 < nbh:
                hnd[bh + 1] = loads(bh + 1)

                def emit_next(b=bh + 1):
                    mids[b] = mid(b, *hnd[b])
            else:
                emit_next = None
            tloop(bh, *mids[bh], emit_next)

    _legalize_waits(nc)
    return nc


_nc_cache = {}


def _get_nc():
    key = (NBH, S, D)
    if key not in _nc_cache:
        _nc_cache[key] = build_nc()
    return _nc_cache[key]


def kernel(query, key, value, ctx_mask):
    q = np.ascontiguousarray(query, dtype=np.float32).reshape(B * H, S, D)
    k = np.ascontiguousarray(key, dtype=np.float32).reshape(B * H, S, D)
    v = np.ascontiguousarray(value, dtype=np.float32).reshape(B * H, S, D)
    cmf = np.ascontiguousarray(ctx_mask, dtype=np.float32)

    in_maps = []
    for c in range(NCORES):
        lo = c * NBH
        in_maps.append({
            "q": q[lo:lo + NBH],
            "k": k[lo:lo + NBH],
            "v": v[lo:lo + NBH],
            "cm": cmf[(lo // H)],
        })
    nc = _get_nc()
    res = run_bass_kernel_spmd(nc, in_maps, list(range(NCORES)))
    outs = [res.results[c]["out"] for c in range(NCORES)]
    return np.concatenate(outs, axis=0).reshape(B, H, S, D).astype(np.float32)


# revision 13
# speedup vs baseline: 1.1528x; 1.0847x over previous
"""Trainium2 Bass kernel for GPT-Neo style causal attention.

reference:
    scores = q @ k.T              (no 1/sqrt(d) scaling), fp32
    scores = where(causal, scores, -inf)
    attn   = softmax(scores, -1)
    attn   = attn * ctx_mask[b, None, None, :]
    out    = attn @ v

Shapes: B=2, H=16, S=2048, D=128 fp32. Sharded over 8 cores by (b*h) —
4 heads per core; each core's heads belong to one batch, so one
ctx_mask row per core.

Per-core algorithm (T-layout softmax: k on partitions, q on free axis):
  - Q,K are cast to fp16 (DVE) and transposed by the DMA XBAR (one
    whole-tensor dma_start_transpose per head-tensor) -> no PE
    transposes, no fp32-family LDWEIGHTS anywhere.
  - per key-block t: scoresT[k,q] = KT_t.T @ QT (fp16, 1 cyc/col; FWL
    weight loads hide under the 512-col streams).
  - one exp() per strip on ScalarE with per-partition bias ln(ctx_mask):
    expT = exp(s - 16 + ln(cm_key)) = exp(s)*cm_key -> bf16 (the
    ctx-mask multiply costs nothing).  Causal diagonal block masked
    post-exp by a 0/1 upper-tri multiply on DVE (bf16; exp stays finite
    for |s| < 104).
  - AV: out_psum[q, 0:129] = sum_kb expT_kb.T @ [V | 1/cm] (bf16,
    fp32 PSUM accum).  Column 128 accumulates exp*cm*(1/cm) = exp,
    i.e. the pre-ctx-mask softmax denominator -> reciprocal + scale.
  - cm clamped at 1e-30 so cm=0 stays exact.

Scheduling: per-head input loads are serialized by a single-buffered
stage pool (full DMA bandwidth for the head at the front), and each
head's cast/transpose/vp prologue is emitted one head ahead of the
previous head's compute loop so the Tile scheduler overlaps them.
A dummy bf16 matmul burst warms the PE HAM clock gate to 2.4 GHz while
the first inputs load.
"""

from contextlib import ExitStack

import numpy as np

import concourse.bass as bass
import concourse.mybir as mybir
import concourse.tile as tile
from concourse.bass_utils import run_bass_kernel_spmd
from concourse.masks import make_upper_triangular

F32 = mybir.dt.float32
F16 = mybir.dt.float16
BF16 = mybir.dt.bfloat16

B, H, S, D = 2, 16, 2048, 128
NCORES = 8
NBH = (B * H) // NCORES  # heads per core


def _legalize_waits(nc):
    """This container's walrus accepts at most 1 sync wait per instruction
    (2 for EventSemaphore). Hoist extra waits onto same-engine NoOps
    inserted immediately before the offending instruction (semantically
    identical: all waits still complete before it executes)."""
    n = 0
    ctr = [0]
    for f in nc.m.functions:
        for bb in f.blocks:
            out = []
            dirty = False
            for inst in bb.instructions:
                si = inst.sync_info
                cap = 2 if isinstance(inst, mybir.InstEventSemaphore) else 1
                if si is not None and len(si.on_wait) > cap:
                    waits = list(si.on_wait)
                    extra, keep = waits[:-cap], waits[-cap:]
                    for w in extra:
                        ctr[0] += 1
                        nop = mybir.InstNoOp(
                            name=f"waitsplit-{ctr[0]}",
                            ins=[],
                            outs=[],
                            engine=inst.engine,
                            sync_info=mybir.SyncInfo(on_wait=[w], on_update=[]),
                        )
                        nc.register_instruction(nop, overwrite=True)
                        out.append(nop)
                    inst.sync_info = mybir.SyncInfo(
                        on_wait=keep, on_update=list(si.on_update)
                    )
                    dirty = True
                    n += 1
                out.append(inst)
            if dirty:
                bb.instructions = out
    return n


def build_nc(nbh=NBH, s=S, d=D, num_devices=NCORES):
    SB = s // 128  # 128-row blocks along the sequence
    nc = bass.Bass("TRN2", target_bir_lowering=False, debug=False,
                   num_devices=num_devices)
    q = nc.dram_tensor("q", [nbh, s, d], F32, kind="ExternalInput")
    k = nc.dram_tensor("k", [nbh, s, d], F32, kind="ExternalInput")
    v = nc.dram_tensor("v", [nbh, s, d], F32, kind="ExternalInput")
    cm = nc.dram_tensor("cm", [s], F32, kind="ExternalInput")
    o = nc.dram_tensor("out", [nbh, s, d], F32, kind="ExternalOutput")

    EXPFN = mybir.ActivationFunctionType.Exp
    LNFN = mybir.ActivationFunctionType.Ln

    with tile.TileContext(nc) as tc, ExitStack() as ctx:
        consts = ctx.enter_context(tc.tile_pool(name="consts", bufs=1))
        h16 = ctx.enter_context(tc.tile_pool(name="h16", bufs=2))
        tpool = ctx.enter_context(tc.tile_pool(name="tpool", bufs=2))
        vpool = ctx.enter_context(tc.tile_pool(name="vpool", bufs=2))
        epool = ctx.enter_context(tc.tile_pool(name="epool", bufs=1))
        opool = ctx.enter_context(tc.tile_pool(name="opool", bufs=2))
        small = ctx.enter_context(tc.tile_pool(name="small", bufs=4))
        psum = ctx.enter_context(tc.tile_pool(name="psum", bufs=2, space="PSUM"))
        psav = ctx.enter_context(tc.tile_pool(name="psav", bufs=2, space="PSUM"))

        # 0/1 upper-triangular (incl diag) keep-mask for the causal
        # diagonal block, applied to expT (post-exp) in bf16.
        tri32 = consts.tile([128, 128], F32)
        make_upper_triangular(nc, tri32, val=1.0, diag=True)
        tri = consts.tile([128, 128], BF16)
        nc.vector.tensor_copy(tri, tri32)

        # ctx-mask pipeline: cmc = max(cm, 1e-30); lncm = ln(cmc) - 16
        # (exp bias); invc = 1/cmc in bf16 (denominator column of V')
        cmt = consts.tile([128, SB], F32)
        nc.sync.dma_start(out=cmt, in_=cm.ap().rearrange("(sb p) -> p sb", p=128))
        cmc = consts.tile([128, SB], F32)
        nc.vector.tensor_scalar_max(cmc, cmt, 1e-30)
        lncm = consts.tile([128, SB], F32)
        nc.scalar.activation(lncm, cmc, LNFN)
        nc.vector.tensor_scalar_add(lncm, lncm, -16.0)
        invc = consts.tile([128, SB], F32)
        nc.vector.reciprocal(invc, cmc)
        invcb = consts.tile([128, SB], BF16)
        nc.vector.tensor_copy(invcb, invc)

        # Dummy bf16 matmuls (values irrelevant) to warm the PE clock gate
        # while the first input DMAs + casts + transposes are in flight.
        wpw = consts.tile([128, 128], BF16)
        nc.vector.memset(wpw, 1.0)
        wps = psav.tile([128, 256], F32, tag="av")
        for _ in range(150):
            nc.tensor.matmul(wps[:, 0:128], wpw, wpw, start=True, stop=True)

        qap, kap, vap, oap = q.ap(), k.ap(), v.ap(), o.ap()

        def loads(bh):
            # SWDGE (gpsimd) DMAs cast in flight: fp32 HBM -> fp16/bf16
            # SBUF directly, no fp32 staging and no DVE cast pass.
            q16 = h16.tile([128, SB, d], F16, tag="q16")
            k16 = h16.tile([128, SB, d], F16, tag="k16")
            vp = vpool.tile([128, SB, d + 1], BF16, tag="vp")
            nc.gpsimd.dma_start(out=q16, in_=qap[bh].rearrange("(sb p) d -> p sb d", p=128))
            nc.gpsimd.dma_start(out=k16, in_=kap[bh].rearrange("(sb p) d -> p sb d", p=128))
            nc.gpsimd.dma_start(out=vp[:, :, 0:d],
                                in_=vap[bh].rearrange("(sb p) d -> p sb d", p=128))
            return q16, k16, vp

        def mid(bh, q16, k16, vp):
            # whole-tensor DMA XBAR transposes: qt[dcol, sb, qrow] = Q^T.
            # For head 0 (exposed lead-in) run the two descriptor
            # generations on both HWDGE queues in parallel.
            qt = tpool.tile([128, SB, 128], F16, tag="qt")
            kt = tpool.tile([128, SB, 128], F16, tag="kt")
            nc.sync.dma_start_transpose(out=qt, in_=q16)
            if bh == 0:
                nc.scalar.dma_start_transpose(out=kt, in_=k16)
            else:
                nc.sync.dma_start_transpose(out=kt, in_=k16)
            # denominator column of V' = [V | 1/cm]
            nc.vector.tensor_copy(vp[:, :, d], invcb)
            return qt, kt, vp

        def tloop(bh, qt, kt, vp, emit_next):
            expT = epool.tile([128, SB, s], BF16, tag="expT",
                              name=f"expT_{bh}")
            ostage = opool.tile([128, SB, d], F32, tag="ostage")

            def av_block(qb):
                av = psav.tile([128, 256], F32, tag="av")
                for kb in range(qb + 1):
                    nc.tensor.matmul(
                        av[:, 0:d + 1],
                        expT[:, kb, qb * 128:(qb + 1) * 128],
                        vp[:, kb, :],
                        start=(kb == 0),
                        stop=(kb == qb),
                    )
                rec = small.tile([128, 1], F32, tag="rec")
                nc.vector.reciprocal(rec, av[:, d:d + 1])
                nc.vector.tensor_scalar_mul(ostage[:, qb, :], av[:, 0:d], rec)

            # scores strips capped at 1536 cols (3 PSUM banks) so two strip
            # slots + the av pool fit in the 8 PSUM banks; the long
            # strips (t < 4) are split into two slots/exps.
            for t in range(SB):
                for (lo, hi) in (((t * 128) // 512 * 512,
                                  min(((t * 128) // 512 * 512) + 1536, s)),
                                 (min(((t * 128) // 512 * 512) + 1536, s), s)):
                    if lo >= hi:
                        continue
                    sc = psum.tile([128, 1536], F32, tag="ps")
                    qstart = max(t * 128, lo)
                    while qstart < hi:
                        seg = min(512 - (qstart % 512), hi - qstart)
                        b0, b1 = qstart // 128, (qstart + seg) // 128
                        nc.tensor.matmul(
                            sc[:, qstart - lo:qstart - lo + seg],
                            kt[:, t, :],
                            qt[:, b0:b1, :],
                            start=True,
                            stop=True,
                        )
                        qstart += seg
                    q0 = max(t * 128, lo)
                    # exp(s - 16 + ln(cm_key)) -> bf16
                    nc.scalar.activation(expT[:, t, q0:hi], sc[:, q0 - lo:hi - lo],
                                         EXPFN, bias=lncm[:, t:t + 1])
                # causal-mask the diagonal block post-exp (0/1 multiply);
                # only the last (kb==qb) AV pair of av_block(t) waits on it
                nc.vector.tensor_mul(expT[:, t, t * 128:(t + 1) * 128],
                                     expT[:, t, t * 128:(t + 1) * 128], tri)
                if t >= 1:
                    av_block(t - 1)  # one step behind so PE never waits on exp
                if t == 5 and emit_next is not None:
                    # next head's cast/transpose/vp prologue, emitted
                    # mid-loop: its input loads have completed by now, so
                    # it slots into engine queues without blocking them
                    emit_next()
            av_block(SB - 1)

            # chunked stores on the GpSimd SWDGE queue: all but the last
            # chunk overlap compute.  3 chunks keep the per-head DMA
            # count at exactly 8 = the Tile DMA sem-lane count, so each
            # DMA's lane predecessor is its own counterpart one head
            # earlier (long completed) instead of an unrelated late DMA.
            for g0, gs in ((0, 6), (6, 5), (11, 5)):
                nc.gpsimd.dma_start(
                    out=oap[bh][g0 * 128:(g0 + gs) * 128].rearrange(
                        "(sb p) d -> p sb d", p=128),
                    in_=ostage[:, g0:g0 + gs, :],
                )

        hnd = {0: loads(0)}
        mids = {0: mid(0, *hnd[0])}
        for bh in range(nbh):
            if bh + 1 < nbh:
                hnd[bh + 1] = loads(bh + 1)

                def emit_next(b=bh + 1):
                    mids[b] = mid(b, *hnd[b])
            else:
                emit_next = None
            tloop(bh, *mids[bh], emit_next)

    _legalize_waits(nc)
    return nc


_nc_cache = {}


def _get_nc():
    key = (NBH, S, D)
    if key not in _nc_cache:
        _nc_cache[key] = build_nc()
    return _nc_cache[key]


def kernel(query, key, value, ctx_mask):
    q = np.ascontiguousarray(query, dtype=np.float32).reshape(B * H, S, D)
    k = np.ascontiguousarray(key, dtype=np.float32).reshape(B * H, S, D)
    v = np.ascontiguousarray(value, dtype=np.float32).reshape(B * H, S, D)
    cmf = np.ascontiguousarray(ctx_mask, dtype=np.float32)

    in_maps = []
    for c in range(NCORES):
        lo = c * NBH
        in_maps.append({
            "q": q[lo:lo + NBH],
            "k": k[lo:lo + NBH],
            "v": v[lo:lo + NBH],
            "cm": cmf[(lo // H)],
        })
    nc = _get_nc()
    res = run_bass_kernel_spmd(nc, in_maps, list(range(NCORES)))
    outs = [res.results[c]["out"] for c in range(NCORES)]
    return np.concatenate(outs, axis=0).reshape(B, H, S, D).astype(np.float32)


# revision 15
# speedup vs baseline: 1.1669x; 1.0123x over previous
"""Trainium2 Bass kernel for GPT-Neo style causal attention.

reference:
    scores = q @ k.T              (no 1/sqrt(d) scaling), fp32
    scores = where(causal, scores, -inf)
    attn   = softmax(scores, -1)
    attn   = attn * ctx_mask[b, None, None, :]
    out    = attn @ v

Shapes: B=2, H=16, S=2048, D=128 fp32. Sharded over 8 cores by (b*h) —
4 heads per core; each core's heads belong to one batch, so one
ctx_mask row per core.

Per-core algorithm (T-layout softmax: k on partitions, q on free axis):
  - Q,K are cast to fp16 (DVE) and transposed by the DMA XBAR (one
    whole-tensor dma_start_transpose per head-tensor) -> no PE
    transposes, no fp32-family LDWEIGHTS anywhere.
  - per key-block t: scoresT[k,q] = KT_t.T @ QT (fp16, 1 cyc/col; FWL
    weight loads hide under the 512-col streams).
  - one exp() per strip on ScalarE with per-partition bias ln(ctx_mask):
    expT = exp(s - 16 + ln(cm_key)) = exp(s)*cm_key -> bf16 (the
    ctx-mask multiply costs nothing).  Causal diagonal block masked
    post-exp by a 0/1 upper-tri multiply on DVE (bf16; exp stays finite
    for |s| < 104).
  - AV: out_psum[q, 0:129] = sum_kb expT_kb.T @ [V | 1/cm] (bf16,
    fp32 PSUM accum).  Column 128 accumulates exp*cm*(1/cm) = exp,
    i.e. the pre-ctx-mask softmax denominator -> reciprocal + scale.
  - cm clamped at 1e-30 so cm=0 stays exact.

Scheduling: per-head input loads are serialized by a single-buffered
stage pool (full DMA bandwidth for the head at the front), and each
head's cast/transpose/vp prologue is emitted one head ahead of the
previous head's compute loop so the Tile scheduler overlaps them.
A dummy bf16 matmul burst warms the PE HAM clock gate to 2.4 GHz while
the first inputs load.
"""

from contextlib import ExitStack

import numpy as np

import concourse.bass as bass
import concourse.mybir as mybir
import concourse.tile as tile
from concourse.bass_utils import run_bass_kernel_spmd
from concourse.masks import make_upper_triangular

F32 = mybir.dt.float32
F16 = mybir.dt.float16
BF16 = mybir.dt.bfloat16

B, H, S, D = 2, 16, 2048, 128
NCORES = 8
NBH = (B * H) // NCORES  # heads per core


def _legalize_waits(nc):
    """This container's walrus accepts at most 1 sync wait per instruction
    (2 for EventSemaphore). Hoist extra waits onto same-engine NoOps
    inserted immediately before the offending instruction (semantically
    identical: all waits still complete before it executes)."""
    n = 0
    ctr = [0]
    for f in nc.m.functions:
        for bb in f.blocks:
            out = []
            dirty = False
            for inst in bb.instructions:
                si = inst.sync_info
                cap = 2 if isinstance(inst, mybir.InstEventSemaphore) else 1
                if si is not None and len(si.on_wait) > cap:
                    waits = list(si.on_wait)
                    extra, keep = waits[:-cap], waits[-cap:]
                    for w in extra:
                        ctr[0] += 1
                        nop = mybir.InstNoOp(
                            name=f"waitsplit-{ctr[0]}",
                            ins=[],
                            outs=[],
                            engine=inst.engine,
                            sync_info=mybir.SyncInfo(on_wait=[w], on_update=[]),
                        )
                        nc.register_instruction(nop, overwrite=True)
                        out.append(nop)
                    inst.sync_info = mybir.SyncInfo(
                        on_wait=keep, on_update=list(si.on_update)
                    )
                    dirty = True
                    n += 1
                out.append(inst)
            if dirty:
                bb.instructions = out
    return n


def build_nc(nbh=NBH, s=S, d=D, num_devices=NCORES):
    SB = s // 128  # 128-row blocks along the sequence
    nc = bass.Bass("TRN2", target_bir_lowering=False, debug=False,
                   num_devices=num_devices)
    q = nc.dram_tensor("q", [nbh, s, d], F32, kind="ExternalInput")
    k = nc.dram_tensor("k", [nbh, s, d], F32, kind="ExternalInput")
    v = nc.dram_tensor("v", [nbh, s, d], F32, kind="ExternalInput")
    cm = nc.dram_tensor("cm", [s], F32, kind="ExternalInput")
    o = nc.dram_tensor("out", [nbh, s, d], F32, kind="ExternalOutput")

    EXPFN = mybir.ActivationFunctionType.Exp
    LNFN = mybir.ActivationFunctionType.Ln

    with tile.TileContext(nc) as tc, ExitStack() as ctx:
        consts = ctx.enter_context(tc.tile_pool(name="consts", bufs=1))
        h16 = ctx.enter_context(tc.tile_pool(name="h16", bufs=2))
        tpool = ctx.enter_context(tc.tile_pool(name="tpool", bufs=2))
        vpool = ctx.enter_context(tc.tile_pool(name="vpool", bufs=2))
        epool = ctx.enter_context(tc.tile_pool(name="epool", bufs=1))
        opool = ctx.enter_context(tc.tile_pool(name="opool", bufs=2))
        small = ctx.enter_context(tc.tile_pool(name="small", bufs=4))
        psum = ctx.enter_context(tc.tile_pool(name="psum", bufs=2, space="PSUM"))
        psav = ctx.enter_context(tc.tile_pool(name="psav", bufs=2, space="PSUM"))

        # 0/1 upper-triangular (incl diag) keep-mask for the causal
        # diagonal block, applied to expT (post-exp) in bf16.
        tri32 = consts.tile([128, 128], F32)
        make_upper_triangular(nc, tri32, val=1.0, diag=True)
        tri = consts.tile([128, 128], BF16)
        nc.vector.tensor_copy(tri, tri32)

        # ctx-mask pipeline: cmc = max(cm, 1e-30); lncm = ln(cmc) - 16
        # (exp bias); invc = 1/cmc in bf16 (denominator column of V')
        cmt = consts.tile([128, SB], F32)
        nc.sync.dma_start(out=cmt, in_=cm.ap().rearrange("(sb p) -> p sb", p=128))
        cmc = consts.tile([128, SB], F32)
        nc.vector.tensor_scalar_max(cmc, cmt, 1e-30)
        lncm = consts.tile([128, SB], F32)
        nc.scalar.activation(lncm, cmc, LNFN)
        nc.vector.tensor_scalar_add(lncm, lncm, -16.0)
        invc = consts.tile([128, SB], F32)
        nc.vector.reciprocal(invc, cmc)
        invcb = consts.tile([128, SB], BF16)
        nc.vector.tensor_copy(invcb, invc)

        # Dummy bf16 matmuls (values irrelevant) to warm the PE clock gate
        # while the first input DMAs + casts + transposes are in flight.
        wpw = consts.tile([128, 128], BF16)
        nc.vector.memset(wpw, 1.0)
        wps = psav.tile([128, 256], F32, tag="av")
        for _ in range(150):
            nc.tensor.matmul(wps[:, 0:128], wpw, wpw, start=True, stop=True)

        qap, kap, vap, oap = q.ap(), k.ap(), v.ap(), o.ap()

        def loads(bh):
            # SWDGE (gpsimd) DMAs cast in flight: fp32 HBM -> fp16/bf16
            # SBUF directly, no fp32 staging and no DVE cast pass.
            # Only Q,K here: they gate the transposes -> QK critical path.
            q16 = h16.tile([128, SB, d], F16, tag="q16")
            k16 = h16.tile([128, SB, d], F16, tag="k16")
            nc.gpsimd.dma_start(out=q16, in_=qap[bh].rearrange("(sb p) d -> p sb d", p=128))
            nc.gpsimd.dma_start(out=k16, in_=kap[bh].rearrange("(sb p) d -> p sb d", p=128))
            return q16, k16

        def mid(bh, q16, k16):
            # whole-tensor DMA XBAR transposes: qt[dcol, sb, qrow] = Q^T.
            # For head 0 (exposed lead-in) run the two descriptor
            # generations on both HWDGE queues in parallel.
            qt = tpool.tile([128, SB, 128], F16, tag="qt")
            kt = tpool.tile([128, SB, 128], F16, tag="kt")
            nc.sync.dma_start_transpose(out=qt, in_=q16)
            if bh == 0:
                nc.scalar.dma_start_transpose(out=kt, in_=k16)
            else:
                nc.sync.dma_start_transpose(out=kt, in_=k16)
            # V' = [V | 1/cm] bf16: emitted after the transposes so its
            # DMA never precedes them in ring/sem-lane order (it is only
            # needed one strip into the t-loop).
            vp = vpool.tile([128, SB, d + 1], BF16, tag="vp")
            nc.gpsimd.dma_start(out=vp[:, :, 0:d],
                                in_=vap[bh].rearrange("(sb p) d -> p sb d", p=128))
            nc.vector.tensor_copy(vp[:, :, d], invcb)
            return qt, kt, vp

        def tloop(bh, qt, kt, vp, emit_next):
            expT = epool.tile([128, SB, s], BF16, tag="expT",
                              name=f"expT_{bh}")
            ostage = opool.tile([128, SB, d], F32, tag="ostage")

            def av_block(qb):
                av = psav.tile([128, 256], F32, tag="av")
                for kb in range(qb + 1):
                    nc.tensor.matmul(
                        av[:, 0:d + 1],
                        expT[:, kb, qb * 128:(qb + 1) * 128],
                        vp[:, kb, :],
                        start=(kb == 0),
                        stop=(kb == qb),
                    )
                rec = small.tile([128, 1], F32, tag="rec")
                nc.vector.reciprocal(rec, av[:, d:d + 1])
                nc.vector.tensor_scalar_mul(ostage[:, qb, :], av[:, 0:d], rec)

            # scores strips capped at 1536 cols (3 PSUM banks) so two strip
            # slots + the av pool fit in the 8 PSUM banks; the long
            # strips (t < 4) are split into two slots/exps.
            for t in range(SB):
                for (lo, hi) in (((t * 128) // 512 * 512,
                                  min(((t * 128) // 512 * 512) + 1536, s)),
                                 (min(((t * 128) // 512 * 512) + 1536, s), s)):
                    if lo >= hi:
                        continue
                    sc = psum.tile([128, 1536], F32, tag="ps")
                    qstart = max(t * 128, lo)
                    while qstart < hi:
                        seg = min(512 - (qstart % 512), hi - qstart)
                        b0, b1 = qstart // 128, (qstart + seg) // 128
                        nc.tensor.matmul(
                            sc[:, qstart - lo:qstart - lo + seg],
                            kt[:, t, :],
                            qt[:, b0:b1, :],
                            start=True,
                            stop=True,
                        )
                        qstart += seg
                    q0 = max(t * 128, lo)
                    # exp(s - 16 + ln(cm_key)) -> bf16
                    nc.scalar.activation(expT[:, t, q0:hi], sc[:, q0 - lo:hi - lo],
                                         EXPFN, bias=lncm[:, t:t + 1])
                # causal-mask the diagonal block post-exp (0/1 multiply);
                # only the last (kb==qb) AV pair of av_block(t) waits on it
                nc.vector.tensor_mul(expT[:, t, t * 128:(t + 1) * 128],
                                     expT[:, t, t * 128:(t + 1) * 128], tri)
                if t >= 1:
                    av_block(t - 1)  # one step behind so PE never waits on exp
                if t == 5 and emit_next is not None:
                    # next head's cast/transpose/vp prologue, emitted
                    # mid-loop: its input loads have completed by now, so
                    # it slots into engine queues without blocking them
                    emit_next()
            av_block(SB - 1)

            # chunked stores on the GpSimd SWDGE queue: all but the last
            # chunk overlap compute.  3 chunks keep the per-head DMA
            # count at exactly 8 = the Tile DMA sem-lane count, so each
            # DMA's lane predecessor is its own counterpart one head
            # earlier (long completed) instead of an unrelated late DMA.
            for g0, gs in ((0, 7), (7, 7), (14, 2)):
                nc.gpsimd.dma_start(
                    out=oap[bh][g0 * 128:(g0 + gs) * 128].rearrange(
                        "(sb p) d -> p sb d", p=128),
                    in_=ostage[:, g0:g0 + gs, :],
                )

        hnd = {0: loads(0)}
        mids = {0: mid(0, *hnd[0])}
        for bh in range(nbh):
            if bh + 1 < nbh:
                hnd[bh + 1] = loads(bh + 1)

                def emit_next(b=bh + 1):
                    mids[b] = mid(b, *hnd[b])
            else:
                emit_next = None
            tloop(bh, *mids[bh], emit_next)

    _legalize_waits(nc)
    return nc


_nc_cache = {}


def _get_nc():
    key = (NBH, S, D)
    if key not in _nc_cache:
        _nc_cache[key] = build_nc()
    return _nc_cache[key]


def kernel(query, key, value, ctx_mask):
    q = np.ascontiguousarray(query, dtype=np.float32).reshape(B * H, S, D)
    k = np.ascontiguousarray(key, dtype=np.float32).reshape(B * H, S, D)
    v = np.ascontiguousarray(value, dtype=np.float32).reshape(B * H, S, D)
    cmf = np.ascontiguousarray(ctx_mask, dtype=np.float32)

    in_maps = []
    for c in range(NCORES):
        lo = c * NBH
        in_maps.append({
            "q": q[lo:lo + NBH],
            "k": k[lo:lo + NBH],
            "v": v[lo:lo + NBH],
            "cm": cmf[(lo // H)],
        })
    nc = _get_nc()
    res = run_bass_kernel_spmd(nc, in_maps, list(range(NCORES)))
    outs = [res.results[c]["out"] for c in range(NCORES)]
    return np.concatenate(outs, axis=0).reshape(B, H, S, D).astype(np.float32)
